# revision 16
# baseline (speedup 1.0000x reference)
"""Paged GQA chunked-prefill attention for 8 Trainium2 NeuronCores.

Problem (hardcoded): B=4 seqs x Q=256 new tokens, H=32 query heads, KVH=8 kv
heads (GQA group G=4), D=128 head dim, paged KV cache of 512 blocks x 16
tokens, per-seq lengths in seq_lens (clamped to >= Q), causal masking.

Sharding: tensor-parallel over heads. Core h gets kv head h and query heads
h*4..h*4+3; block_tables/seq_lens are resolved host-side while packing the
shards; the output is all-gathered host-side over the hidden dim.

v3 design:
- All matmul operands bf16 (K, Q*SCALE, V, U); scores fp32 in PSUM.
- Softmax denominator: U accumulated per chunk into a per-seq bf16
  accumulator on VectorE; [128,1024] accumulator DMA'd out, 128-way
  partition sum + divide on the host. No reciprocal/broadcast epilogue.
- One exp ACTIVATE per chunk covering exactly the live (unmasked) columns;
  QK/PV/lacc are restricted to live columns too, so the causal-dead
  triangle costs nothing on any engine.
- Global 1-chunk software pipeline across sequences: emit score(j) then
  consume(j-1), so the PE does the next chunk's QK while ScalarE runs exp
  and sequence boundaries don't drain the pipe.
- PSUM: 2 double-buffered [128,1024] score tiles + 2 [128,1024] O
  accumulators = 8 banks exactly.
- ScalarE exp-table load and PE HAM warmup fire at t~0 on memset data;
  sequences run longest-first; O^T halves are evacuated as soon as their
  PV chain stops; lacc leaves on the gpsimd DMA queue.
"""
import math

import ml_dtypes
import numpy as np

import concourse.mybir as mybir
import concourse.tile as tile
from concourse import bacc
from concourse.bass_utils import run_bass_kernel_spmd

B, Q, H, D = 4, 256, 32, 128
KVH = 8
G = H // KVH
BLOCK = 16
NB = 128
KV = NB * BLOCK
NUM_BLOCKS = B * NB
SCALE = 1.0 / math.sqrt(D)
N_CORES = 8
CHUNK = 128
QCOLS = G * Q  # 1024 q columns per sequence per core
NHALF = 512

F32 = mybir.dt.float32
BF16 = mybir.dt.bfloat16
NEG = -1.0e9
WARMUP_MM = 6


def _plan(seq_lens):
    """Per-seq chunk counts, offsets, and boundary-chunk mask tiles."""
    L = np.maximum(np.asarray(seq_lens, dtype=np.int64), Q)
    cb = [int((int(Lb) + CHUNK - 1) // CHUNK) for Lb in L]
    offs = np.concatenate([[0], np.cumsum(cb)]).astype(int)
    masked = []  # list of (b, c, mask[128,256])
    t = np.arange(Q)
    p = np.arange(CHUNK)
    for b in range(B):
        Lb = int(L[b])
        for c in range(cb[b]):
            if c * CHUNK + CHUNK - 1 > Lb - Q:
                kvpos = c * CHUNK + p
                m = np.where(
                    kvpos[:, None] > (Lb - Q) + t[None, :], NEG, 0.0
                ).astype(np.float32)
                masked.append((b, c, m))
    return L, cb, offs, masked


def _build(seq_lens):
    L, cb, offs, masked = _plan(seq_lens)
    C = int(offs[-1])
    nmask = len(masked)
    border = sorted(range(B), key=lambda b: -cb[b])  # longest first
    order = sorted(
        range(len(masked)), key=lambda i: (border.index(masked[i][0]), masked[i][1])
    )
    masked = [masked[i] for i in order]
    mask_np = np.concatenate([m for _, _, m in masked], axis=1).astype(
        ml_dtypes.bfloat16
    )  # [128, nm*256]; 0/-1e9 are bf16-exact
    mask_idx = {(b, c): i for i, (b, c, _) in enumerate(masked)}
    identb_np = np.eye(CHUNK, dtype=ml_dtypes.bfloat16)

    nc = bacc.Bacc(
        "TRN2", target_bir_lowering=False, debug=False, num_devices=N_CORES
    )
    kt_d = nc.dram_tensor("kt", [D, C * CHUNK], BF16, kind="ExternalInput")
    v_d = nc.dram_tensor("v", [CHUNK, C * CHUNK], BF16, kind="ExternalInput")
    qt_d = nc.dram_tensor("qt", [D, B * QCOLS], BF16, kind="ExternalInput")
    out_d = nc.dram_tensor("out", [B, D, QCOLS], F32, kind="ExternalOutput")
    lacc_d = nc.dram_tensor("lacc", [B, CHUNK, QCOLS], BF16, kind="ExternalOutput")
    mask_d = nc.inline_tensor(mask_np, name="mask_const")
    identb_d = nc.inline_tensor(identb_np, name="identb_const")

    exp = mybir.ActivationFunctionType.Exp

    def states_of(b, c):
        # per half: (state, qlo_t) where qlo_t = dead leading t-cols
        out = []
        for n in range(2):
            lo = int(L[b]) - Q + n * CHUNK
            qlo = max(0, c * CHUNK - lo)
            if qlo >= CHUNK:
                out.append(("skip", CHUNK))
            elif c * CHUNK + CHUNK - 1 > lo:
                out.append(("mask", qlo))
            else:
                out.append(("clear", 0))
        return out

    with tile.TileContext(nc) as tc:
        with (
            tc.tile_pool(name="sbin", bufs=1) as sbin,
            tc.tile_pool(name="sbu", bufs=4) as sbu,
            tc.tile_pool(name="sbe", bufs=2) as sbe,
            tc.tile_pool(name="ps_s", bufs=2, space="PSUM") as ps_s,
            tc.tile_pool(name="ps_o", bufs=2, space="PSUM") as ps_o,
        ):
            # ─── t~0 warmup: exp table load + PE HAM ramp on memset data ──
            warm = sbin.tile([CHUNK, NHALF], BF16, tag="warm")
            nc.vector.memset(warm[:], 0.0)
            u_dead = sbu.tile([CHUNK, QCOLS], BF16, tag="u")
            nc.scalar.activation(u_dead[:, 0:CHUNK], warm[:, 0:CHUNK], exp)
            s_warm = ps_s.tile([CHUNK, QCOLS], F32, tag="s")
            for w in range(WARMUP_MM):
                nc.tensor.matmul(
                    s_warm[:, 0:NHALF] if w % 2 == 0 else s_warm[:, NHALF:QCOLS],
                    warm[:, 0:CHUNK],
                    warm[:],
                    start=True,
                    stop=True,
                )

            # ─── input DMAs: first-QK gates first, then streaming ─────────
            b0 = border[0]
            kt_t = [None] * B
            qt_t = [None] * B
            v_t = [None] * B
            w0 = cb[b0] * CHUNK
            o0 = offs[b0] * CHUNK
            kt_first = sbin.tile([D, w0], BF16, tag=f"kt{b0}")
            v_first = sbin.tile([CHUNK, w0], BF16, tag=f"v{b0}")
            qt_first = sbin.tile([D, QCOLS], BF16, tag=f"qt{b0}")
            # sync (HWDGE) queue: exactly what the first QK needs, smallest
            # pieces first
            # the three tensors gating the first exp go out on three parallel
            # DMA queues (sync + scalar are fast HWDGE rings; gpsimd SWDGE)
            nc.sync.dma_start(kt_first[:, 0:CHUNK], kt_d.ap()[:, o0 : o0 + CHUNK])
            nc.scalar.dma_start(
                qt_first[:, 0:NHALF],
                qt_d.ap()[:, b0 * QCOLS : b0 * QCOLS + NHALF],
            )
            nc.scalar.dma_start(
                qt_first[:, NHALF:QCOLS],
                qt_d.ap()[:, b0 * QCOLS + NHALF : (b0 + 1) * QCOLS],
            )
            # v chunk 0-1 (needed one pipeline stage later) on gpsimd
            head = min(2 * CHUNK, w0)
            nc.gpsimd.dma_start(v_first[:, 0:head], v_d.ap()[:, o0 : o0 + head])
            kcut, vcut = CHUNK, head
            while kcut < w0 or vcut < w0:
                khi = min(kcut + 4 * CHUNK, w0)
                if khi > kcut:
                    nc.sync.dma_start(
                        kt_first[:, kcut:khi], kt_d.ap()[:, o0 + kcut : o0 + khi]
                    )
                    kcut = khi
                vhi = min(vcut + 4 * CHUNK, w0)
                if vhi > vcut:
                    nc.sync.dma_start(
                        v_first[:, vcut:vhi], v_d.ap()[:, o0 + vcut : o0 + vhi]
                    )
                    vcut = vhi
            kt_t[b0], v_t[b0], qt_t[b0] = kt_first, v_first, qt_first

            identb = sbin.tile([CHUNK, CHUNK], BF16, tag="identb")
            nc.gpsimd.dma_start(identb[:], identb_d.ap())

            # filler (short, masked-heavy) sequences first on gpsimd — they
            # interleave into the long sequence early; the second-longest
            # last (it runs at the end)
            for b in border[2:] + border[1:2]:
                w = cb[b] * CHUNK
                ob = offs[b] * CHUNK
                kt = sbin.tile([D, w], BF16, tag=f"kt{b}")
                vt = sbin.tile([CHUNK, w], BF16, tag=f"v{b}")
                qt = sbin.tile([D, QCOLS], BF16, tag=f"qt{b}")
                nc.gpsimd.dma_start(kt[:], kt_d.ap()[:, ob : ob + w])
                nc.gpsimd.dma_start(vt[:], v_d.ap()[:, ob : ob + w])
                nc.gpsimd.dma_start(
                    qt[:], qt_d.ap()[:, b * QCOLS : (b + 1) * QCOLS]
                )
                kt_t[b], v_t[b], qt_t[b] = kt, vt, qt

            # masks on the scalar HWDGE queue: needed once fillers start
            masks = sbin.tile([CHUNK, max(nmask, 1) * Q], BF16, tag="masks")
            if nmask:
                mid = (nmask // 2) * Q
                if mid:
                    nc.scalar.dma_start(masks[:, 0:mid], mask_d.ap()[:, 0:mid])
                nc.scalar.dma_start(
                    masks[:, mid : nmask * Q], mask_d.ap()[:, mid : nmask * Q]
                )

            # ─── main loop: global 1-chunk software pipeline ──────────────
            # Interleave: the two longest sequences run back to back as the
            # spine (ScalarE-paced, mostly-clear chunks); the short
            # masked-heavy sequences (PE-paced) are injected between spine
            # chunks so both engines stay fed. Filler seq i rides spine seq
            # i so the 2-buffer PSUM O pool alternates cleanly: spine0->buf0
            # b0 filler->buf1, spine1->buf0 (spine0 gone), filler1->buf1.
            def weave(spine_b, fill_b, start):
                sp = [(spine_b, c) for c in range(cb[spine_b])]
                fl = [(fill_b, c) for c in range(cb[fill_b])] if fill_b is not None else []
                gap = max(2, (len(sp) - start) // max(len(fl), 1))
                out = []
                fi = 0
                for si, j in enumerate(sp):
                    out.append(j)
                    if si >= start and (si - start) % gap == 0 and fi < len(fl):
                        out.append(fl[fi])
                        fi += 1
                out.extend(fl[fi:])
                return out

            jobs = weave(border[0], border[2] if B > 2 else None, 3) + weave(
                border[1], border[3] if B > 3 else None, 1
            )
            seq_state = {}  # b -> dict(o_ps, lacc, last_n)

            def emit_score(b, c):
                st = states_of(b, c)
                mi = mask_idx.get((b, c))
                if c == 0:
                    seq_state[b] = {
                        "o": ps_o.tile([D, QCOLS], F32, tag="o", name="o_ps"),
                        "lacc": sbin.tile(
                            [CHUNK, QCOLS], BF16, tag=f"lacc{b}", name=f"lacc{b}"
                        ),
                        "last_n": [
                            min(
                                cb[b] - 1,
                                (int(L[b]) - Q + n * CHUNK + CHUNK - 1) // CHUNK,
                            )
                            for n in range(2)
                        ],
                    }
                ss = seq_state[b]
                s = ps_s.tile([CHUNK, QCOLS], F32, tag="s")
                live_lo = QCOLS
                # both QK matmuls first (K chunk stays the stationary
                # operand), then both mask matmuls (identity stationary) —
                # per-bank write order (QK then mask) is preserved by the
                # PE's in-order queue
                for n in range(2):
                    state, qlo = st[n]
                    if state == "skip":
                        continue
                    col0 = n * NHALF + qlo * G
                    live_lo = min(live_lo, col0)
                    hi = (n + 1) * NHALF
                    nc.tensor.matmul(
                        s[:, col0:hi],
                        kt_t[b][:, c * CHUNK : (c + 1) * CHUNK],
                        qt_t[b][:, col0:hi],
                        start=True,
                        stop=state == "clear",
                    )
                for n in range(2):
                    state, qlo = st[n]
                    if state != "mask":
                        continue
                    col0 = n * NHALF + qlo * G
                    hi = (n + 1) * NHALF
                    mb = (
                        masks[
                            :,
                            mi * Q + n * CHUNK + qlo : mi * Q + (n + 1) * CHUNK,
                        ]
                        .unsqueeze(2)
                        .broadcast_to([CHUNK, CHUNK - qlo, G])
                    )
                    nc.tensor.matmul(
                        s[:, col0:hi], identb[:], mb, start=False, stop=True
                    )
                u = sbu.tile([CHUNK, QCOLS], BF16, tag="u")
                live = slice(live_lo, QCOLS)
                nc.scalar.activation(u[:, live], s[:, live], exp)
                if c == 0:
                    nc.vector.tensor_copy(ss["lacc"][:, live], u[:, live])
                else:
                    nc.vector.tensor_add(
                        ss["lacc"][:, live], ss["lacc"][:, live], u[:, live]
                    )
                return st, u

            def evacuate_half(b, n, terminal=False):
                ss = seq_state[b]
                half = slice(n * NHALF, (n + 1) * NHALF)
                osb = sbe.tile([D, NHALF], F32, tag=f"osb{n}")
                if terminal:
                    # ScalarE is idle after the last exp; its copy + its own
                    # HWDGE queue keep the tail off the busy Vector/Sync FIFOs
                    nc.scalar.copy(osb[:], ss["o"][:, half])
                    nc.scalar.dma_start(out_d.ap()[b][:, half], osb[:])
                else:
                    nc.vector.tensor_copy(osb[:], ss["o"][:, half])
                    nc.sync.dma_start(out_d.ap()[b][:, half], osb[:])

            def emit_consume(b, c, st, u, terminal=False):
                ss = seq_state[b]
                for n in range(2):
                    state, qlo = st[n]
                    if state == "skip" or c > ss["last_n"][n]:
                        continue
                    col0 = n * NHALF + qlo * G
                    hi = (n + 1) * NHALF
                    nc.tensor.matmul(
                        ss["o"][:, col0:hi],
                        v_t[b][:, c * CHUNK : (c + 1) * CHUNK],
                        u[:, col0:hi],
                        start=c == 0,
                        stop=c == ss["last_n"][n],
                    )
                    if c == ss["last_n"][n]:
                        evacuate_half(b, n, terminal=terminal and n == 1)
                if c == cb[b] - 1:
                    nc.gpsimd.dma_start(lacc_d.ap()[b], ss["lacc"][:])

            pending = None
            for b, c in jobs:
                st_u = emit_score(b, c)
                if pending is not None:
                    emit_consume(*pending)
                pending = (b, c, *st_u)
            emit_consume(*pending, terminal=True)

    nc.compile()
    return nc, L, cb, offs


def _pack_inputs(query, k_cache, v_cache, block_tables, L, cb, offs):
    """Gather the paged cache and pack per-core shards in device layouts."""
    C = int(offs[-1])
    k_lin = k_cache[block_tables].reshape(B, KV, KVH, D)
    v_lin = v_cache[block_tables].reshape(B, KV, KVH, D)
    kt_all = np.zeros((KVH, D, C * CHUNK), dtype=np.float32)
    v_all = np.zeros((KVH, CHUNK, C * CHUNK), dtype=np.float32)
    for b in range(B):
        Lb, w = int(L[b]), cb[b] * CHUNK
        kk = np.zeros((w, KVH, D), dtype=np.float32)
        kk[:Lb] = k_lin[b, :Lb]
        # [w, KVH, D] -> [KVH, D, w]
        kt_all[:, :, offs[b] * CHUNK : offs[b] * CHUNK + w] = kk.transpose(
            1, 2, 0
        )
        vv = np.zeros((w, KVH, D), dtype=np.float32)
        vv[:Lb] = v_lin[b, :Lb]
        # [cb, 128, KVH, D] -> [KVH, 128, cb, D] -> [KVH, 128, w]
        v_all[:, :, offs[b] * CHUNK : offs[b] * CHUNK + w] = (
            vv.reshape(cb[b], CHUNK, KVH, D)
            .transpose(2, 1, 0, 3)
            .reshape(KVH, CHUNK, w)
        )
    # query [B,Q,H,D] -> [KVH, D, B, Q, G] (t-major, g inner); SCALE folded in
    qt_all = (
        (query * SCALE)
        .transpose(2, 3, 0, 1)
        .reshape(KVH, G, D, B, Q)
        .transpose(0, 2, 3, 4, 1)
        .reshape(KVH, D, B * QCOLS)
    )
    kt_all = kt_all.astype(ml_dtypes.bfloat16)
    v_all = v_all.astype(ml_dtypes.bfloat16)
    qt_all = np.ascontiguousarray(qt_all).astype(ml_dtypes.bfloat16)
    return [
        {
            "kt": np.ascontiguousarray(kt_all[h]),
            "v": np.ascontiguousarray(v_all[h]),
            "qt": qt_all[h],
        }
        for h in range(KVH)
    ]


def _unpack_outputs(results):
    """[B,D,QCOLS] O^T + [B,128,QCOLS] lacc per core -> [B*Q, H*D]."""
    out = np.empty((B * Q, H * D), dtype=np.float32)
    for h, res in enumerate(results):
        l = res["lacc"].astype(np.float32).sum(axis=1)  # [B, QCOLS]
        o = res["out"] / l[:, None, :]  # [B, D, QCOLS]
        o = o.reshape(B, D, Q, G).transpose(0, 2, 3, 1).reshape(B * Q, G * D)
        out[:, h * G * D : (h + 1) * G * D] = o
    return out


def kernel(query, k_cache, v_cache, block_tables, seq_lens):
    query = np.asarray(query, dtype=np.float32)
    k_cache = np.asarray(k_cache, dtype=np.float32)
    v_cache = np.asarray(v_cache, dtype=np.float32)
    block_tables = np.asarray(block_tables, dtype=np.int64)
    nc, L, cb, offs = _build(np.asarray(seq_lens))
    in_maps = _pack_inputs(query, k_cache, v_cache, block_tables, L, cb, offs)
    res = run_bass_kernel_spmd(nc, in_maps, core_ids=list(range(N_CORES)))
    return _unpack_outputs(res.results)


# revision 18
# speedup vs baseline: 1.1044x; 1.1044x over previous
"""Paged GQA chunked-prefill attention for 8 Trainium2 NeuronCores.

Problem (hardcoded): B=4 seqs x Q=256 new tokens, H=32 query heads, KVH=8 kv
heads (GQA group G=4), D=128 head dim, paged KV cache of 512 blocks x 16
tokens, per-seq lengths in seq_lens (clamped to >= Q), causal masking.

Sharding: tensor-parallel over heads. Core h gets kv head h and query heads
h*4..h*4+3; block_tables/seq_lens are resolved host-side while packing the
shards; the output is all-gathered host-side over the hidden dim.

v3 design:
- All matmul operands bf16 (K, Q*SCALE, V, U); scores fp32 in PSUM.
- Softmax denominator: U accumulated per chunk into a per-seq bf16
  accumulator on VectorE; [128,1024] accumulator DMA'd out, 128-way
  partition sum + divide on the host. No reciprocal/broadcast epilogue.
- One exp ACTIVATE per chunk covering exactly the live (unmasked) columns;
  QK/PV/lacc are restricted to live columns too, so the causal-dead
  triangle costs nothing on any engine.
- Global 1-chunk software pipeline across sequences: emit score(j) then
  consume(j-1), so the PE does the next chunk's QK while ScalarE runs exp
  and sequence boundaries don't drain the pipe.
- PSUM: 2 double-buffered [128,1024] score tiles + 2 [128,1024] O
  accumulators = 8 banks exactly.
- ScalarE exp-table load and PE HAM warmup fire at t~0 on memset data;
  sequences run longest-first; O^T halves are evacuated as soon as their
  PV chain stops; lacc leaves on the gpsimd DMA queue.
"""
import math

import ml_dtypes
import numpy as np

import concourse.mybir as mybir
import concourse.tile as tile
from concourse import bacc
from concourse.bass_utils import run_bass_kernel_spmd

B, Q, H, D = 4, 256, 32, 128
KVH = 8
G = H // KVH
BLOCK = 16
NB = 128
KV = NB * BLOCK
NUM_BLOCKS = B * NB
SCALE = 1.0 / math.sqrt(D)
N_CORES = 8
CHUNK = 128
QCOLS = G * Q  # 1024 q columns per sequence per core
NHALF = 512

F32 = mybir.dt.float32
BF16 = mybir.dt.bfloat16
NEG = -1.0e9
WARMUP_MM = 6


def _plan(seq_lens):
    """Per-seq chunk counts, offsets, and boundary-chunk mask tiles."""
    L = np.maximum(np.asarray(seq_lens, dtype=np.int64), Q)
    cb = [int((int(Lb) + CHUNK - 1) // CHUNK) for Lb in L]
    offs = np.concatenate([[0], np.cumsum(cb)]).astype(int)
    masked = []  # list of (b, c, mask[128,256])
    t = np.arange(Q)
    p = np.arange(CHUNK)
    for b in range(B):
        Lb = int(L[b])
        for c in range(cb[b]):
            if c * CHUNK + CHUNK - 1 > Lb - Q:
                kvpos = c * CHUNK + p
                m = np.where(
                    kvpos[:, None] > (Lb - Q) + t[None, :], NEG, 0.0
                ).astype(np.float32)
                masked.append((b, c, m))
    return L, cb, offs, masked


def _build(seq_lens):
    L, cb, offs, masked = _plan(seq_lens)
    C = int(offs[-1])
    nmask = len(masked)
    border = sorted(range(B), key=lambda b: -cb[b])  # longest first
    order = sorted(
        range(len(masked)), key=lambda i: (border.index(masked[i][0]), masked[i][1])
    )
    masked = [masked[i] for i in order]
    mask_np = np.concatenate([m for _, _, m in masked], axis=1).astype(
        ml_dtypes.bfloat16
    )  # [128, nm*256]; 0/-1e9 are bf16-exact
    mask_idx = {(b, c): i for i, (b, c, _) in enumerate(masked)}
    identb_np = np.eye(CHUNK, dtype=ml_dtypes.bfloat16)

    nc = bacc.Bacc(
        "TRN2", target_bir_lowering=False, debug=False, num_devices=N_CORES
    )
    kt_d = nc.dram_tensor("kt", [D, C * CHUNK], BF16, kind="ExternalInput")
    v_d = nc.dram_tensor("v", [CHUNK, C * CHUNK], BF16, kind="ExternalInput")
    qt_d = nc.dram_tensor("qt", [D, B * QCOLS], BF16, kind="ExternalInput")
    out_d = nc.dram_tensor("out", [B, D, QCOLS], F32, kind="ExternalOutput")
    lacc_d = nc.dram_tensor("lacc", [B, CHUNK, QCOLS], BF16, kind="ExternalOutput")
    mask_d = nc.inline_tensor(mask_np, name="mask_const")
    identb_d = nc.inline_tensor(identb_np, name="identb_const")

    exp = mybir.ActivationFunctionType.Exp

    def states_of(b, c):
        # per half: (state, qlo_t) where qlo_t = dead leading t-cols
        out = []
        for n in range(2):
            lo = int(L[b]) - Q + n * CHUNK
            qlo = max(0, c * CHUNK - lo)
            if qlo >= CHUNK:
                out.append(("skip", CHUNK))
            elif c * CHUNK + CHUNK - 1 > lo:
                out.append(("mask", qlo))
            else:
                out.append(("clear", 0))
        return out

    with tile.TileContext(nc) as tc:
        with (
            tc.tile_pool(name="sbin", bufs=1) as sbin,
            tc.tile_pool(name="sbu", bufs=4) as sbu,
            tc.tile_pool(name="sbe", bufs=2) as sbe,
            tc.tile_pool(name="ps_s", bufs=2, space="PSUM") as ps_s,
            tc.tile_pool(name="ps_o", bufs=2, space="PSUM") as ps_o,
        ):
            # ─── t~0 warmup: exp table load + PE HAM ramp on memset data ──
            warm = sbin.tile([CHUNK, NHALF], BF16, tag="warm")
            nc.vector.memset(warm[:], 0.0)
            u_dead = sbu.tile([CHUNK, QCOLS], BF16, tag="u")
            nc.scalar.activation(u_dead[:, 0:CHUNK], warm[:, 0:CHUNK], exp)
            s_warm = ps_s.tile([CHUNK, QCOLS], F32, tag="s")
            for w in range(WARMUP_MM):
                nc.tensor.matmul(
                    s_warm[:, 0:NHALF] if w % 2 == 0 else s_warm[:, NHALF:QCOLS],
                    warm[:, 0:CHUNK],
                    warm[:],
                    start=True,
                    stop=True,
                )

            # ─── input DMAs: first-QK gates first, then streaming ─────────
            b0 = border[0]
            kt_t = [None] * B
            qt_t = [None] * B
            v_t = [None] * B
            w0 = cb[b0] * CHUNK
            o0 = offs[b0] * CHUNK
            kt_first = sbin.tile([D, w0], BF16, tag=f"kt{b0}")
            v_first = sbin.tile([CHUNK, w0], BF16, tag=f"v{b0}")
            qt_first = sbin.tile([D, QCOLS], BF16, tag=f"qt{b0}")
            # sync (HWDGE) queue: exactly what the first QK needs, smallest
            # pieces first
            # the three tensors gating the first exp go out on three parallel
            # DMA queues (sync + scalar are fast HWDGE rings; gpsimd SWDGE)
            nc.sync.dma_start(kt_first[:, 0:CHUNK], kt_d.ap()[:, o0 : o0 + CHUNK])
            nc.scalar.dma_start(
                qt_first[:, 0:NHALF],
                qt_d.ap()[:, b0 * QCOLS : b0 * QCOLS + NHALF],
            )
            nc.scalar.dma_start(
                qt_first[:, NHALF:QCOLS],
                qt_d.ap()[:, b0 * QCOLS + NHALF : (b0 + 1) * QCOLS],
            )
            # v chunk 0-1 (needed one pipeline stage later) on gpsimd
            head = min(2 * CHUNK, w0)
            nc.gpsimd.dma_start(v_first[:, 0:head], v_d.ap()[:, o0 : o0 + head])
            kcut, vcut = CHUNK, head
            while kcut < w0 or vcut < w0:
                khi = min(kcut + 4 * CHUNK, w0)
                if khi > kcut:
                    nc.sync.dma_start(
                        kt_first[:, kcut:khi], kt_d.ap()[:, o0 + kcut : o0 + khi]
                    )
                    kcut = khi
                vhi = min(vcut + 4 * CHUNK, w0)
                if vhi > vcut:
                    nc.sync.dma_start(
                        v_first[:, vcut:vhi], v_d.ap()[:, o0 + vcut : o0 + vhi]
                    )
                    vcut = vhi
            kt_t[b0], v_t[b0], qt_t[b0] = kt_first, v_first, qt_first

            identb = sbin.tile([CHUNK, CHUNK], BF16, tag="identb")
            nc.gpsimd.dma_start(identb[:], identb_d.ap())

            # remaining sequences stream on the gpsimd (SWDGE) queue in
            # processing order
            for b in border[1:]:
                w = cb[b] * CHUNK
                ob = offs[b] * CHUNK
                kt = sbin.tile([D, w], BF16, tag=f"kt{b}")
                vt = sbin.tile([CHUNK, w], BF16, tag=f"v{b}")
                qt = sbin.tile([D, QCOLS], BF16, tag=f"qt{b}")
                nc.gpsimd.dma_start(kt[:], kt_d.ap()[:, ob : ob + w])
                nc.gpsimd.dma_start(vt[:], v_d.ap()[:, ob : ob + w])
                nc.gpsimd.dma_start(
                    qt[:], qt_d.ap()[:, b * QCOLS : (b + 1) * QCOLS]
                )
                kt_t[b], v_t[b], qt_t[b] = kt, vt, qt

            masks = sbin.tile([CHUNK, max(nmask, 1) * Q], BF16, tag="masks")
            if nmask:
                mid = (nmask // 2) * Q
                if mid:
                    nc.sync.dma_start(masks[:, 0:mid], mask_d.ap()[:, 0:mid])
                nc.sync.dma_start(
                    masks[:, mid : nmask * Q], mask_d.ap()[:, mid : nmask * Q]
                )

            # ─── main loop: global 1-chunk software pipeline ──────────────
            jobs = [(b, c) for b in border for c in range(cb[b])]
            seq_state = {}  # b -> dict(o_ps, lacc, last_n)

            def emit_score(b, c):
                st = states_of(b, c)
                mi = mask_idx.get((b, c))
                if c == 0:
                    seq_state[b] = {
                        "o": ps_o.tile([D, QCOLS], F32, tag="o", name="o_ps"),
                        "lacc": sbin.tile(
                            [CHUNK, QCOLS], BF16, tag=f"lacc{b}", name=f"lacc{b}"
                        ),
                        "last_n": [
                            min(
                                cb[b] - 1,
                                (int(L[b]) - Q + n * CHUNK + CHUNK - 1) // CHUNK,
                            )
                            for n in range(2)
                        ],
                    }
                ss = seq_state[b]
                s = ps_s.tile([CHUNK, QCOLS], F32, tag="s")
                live_lo = QCOLS
                # both QK matmuls first (K chunk stays the stationary
                # operand), then both mask matmuls (identity stationary) —
                # per-bank write order (QK then mask) is preserved by the
                # PE's in-order queue
                for n in range(2):
                    state, qlo = st[n]
                    if state == "skip":
                        continue
                    col0 = n * NHALF + qlo * G
                    live_lo = min(live_lo, col0)
                    hi = (n + 1) * NHALF
                    nc.tensor.matmul(
                        s[:, col0:hi],
                        kt_t[b][:, c * CHUNK : (c + 1) * CHUNK],
                        qt_t[b][:, col0:hi],
                        start=True,
                        stop=state == "clear",
                    )
                for n in range(2):
                    state, qlo = st[n]
                    if state != "mask":
                        continue
                    col0 = n * NHALF + qlo * G
                    hi = (n + 1) * NHALF
                    mb = (
                        masks[
                            :,
                            mi * Q + n * CHUNK + qlo : mi * Q + (n + 1) * CHUNK,
                        ]
                        .unsqueeze(2)
                        .broadcast_to([CHUNK, CHUNK - qlo, G])
                    )
                    nc.tensor.matmul(
                        s[:, col0:hi], identb[:], mb, start=False, stop=True
                    )
                u = sbu.tile([CHUNK, QCOLS], BF16, tag="u")
                live = slice(live_lo, QCOLS)
                nc.scalar.activation(u[:, live], s[:, live], exp)
                if c == 0:
                    nc.vector.tensor_copy(ss["lacc"][:, live], u[:, live])
                else:
                    nc.vector.tensor_add(
                        ss["lacc"][:, live], ss["lacc"][:, live], u[:, live]
                    )
                return st, u

            def evacuate_half(b, n, terminal=False):
                ss = seq_state[b]
                half = slice(n * NHALF, (n + 1) * NHALF)
                osb = sbe.tile([D, NHALF], F32, tag=f"osb{n}")
                if terminal:
                    # ScalarE is idle after the last exp; its copy + its own
                    # HWDGE queue keep the tail off the busy Vector/Sync FIFOs
                    nc.scalar.copy(osb[:], ss["o"][:, half])
                    nc.scalar.dma_start(out_d.ap()[b][:, half], osb[:])
                else:
                    nc.vector.tensor_copy(osb[:], ss["o"][:, half])
                    nc.sync.dma_start(out_d.ap()[b][:, half], osb[:])

            def emit_consume(b, c, st, u, terminal=False):
                ss = seq_state[b]
                for n in range(2):
                    state, qlo = st[n]
                    if state == "skip" or c > ss["last_n"][n]:
                        continue
                    col0 = n * NHALF + qlo * G
                    hi = (n + 1) * NHALF
                    nc.tensor.matmul(
                        ss["o"][:, col0:hi],
                        v_t[b][:, c * CHUNK : (c + 1) * CHUNK],
                        u[:, col0:hi],
                        start=c == 0,
                        stop=c == ss["last_n"][n],
                    )
                    if c == ss["last_n"][n]:
                        evacuate_half(b, n, terminal=terminal and n == 1)
                if c == cb[b] - 1:
                    nc.gpsimd.dma_start(lacc_d.ap()[b], ss["lacc"][:])

            pending = None
            for b, c in jobs:
                st_u = emit_score(b, c)
                if pending is not None:
                    emit_consume(*pending)
                pending = (b, c, *st_u)
            emit_consume(*pending, terminal=True)

    nc.compile()
    return nc, L, cb, offs


def _pack_inputs(query, k_cache, v_cache, block_tables, L, cb, offs):
    """Gather the paged cache and pack per-core shards in device layouts."""
    C = int(offs[-1])
    k_lin = k_cache[block_tables].reshape(B, KV, KVH, D)
    v_lin = v_cache[block_tables].reshape(B, KV, KVH, D)
    kt_all = np.zeros((KVH, D, C * CHUNK), dtype=np.float32)
    v_all = np.zeros((KVH, CHUNK, C * CHUNK), dtype=np.float32)
    for b in range(B):
        Lb, w = int(L[b]), cb[b] * CHUNK
        kk = np.zeros((w, KVH, D), dtype=np.float32)
        kk[:Lb] = k_lin[b, :Lb]
        # [w, KVH, D] -> [KVH, D, w]
        kt_all[:, :, offs[b] * CHUNK : offs[b] * CHUNK + w] = kk.transpose(
            1, 2, 0
        )
        vv = np.zeros((w, KVH, D), dtype=np.float32)
        vv[:Lb] = v_lin[b, :Lb]
        # [cb, 128, KVH, D] -> [KVH, 128, cb, D] -> [KVH, 128, w]
        v_all[:, :, offs[b] * CHUNK : offs[b] * CHUNK + w] = (
            vv.reshape(cb[b], CHUNK, KVH, D)
            .transpose(2, 1, 0, 3)
            .reshape(KVH, CHUNK, w)
        )
    # query [B,Q,H,D] -> [KVH, D, B, Q, G] (t-major, g inner); SCALE folded in
    qt_all = (
        (query * SCALE)
        .transpose(2, 3, 0, 1)
        .reshape(KVH, G, D, B, Q)
        .transpose(0, 2, 3, 4, 1)
        .reshape(KVH, D, B * QCOLS)
    )
    kt_all = kt_all.astype(ml_dtypes.bfloat16)
    v_all = v_all.astype(ml_dtypes.bfloat16)
    qt_all = np.ascontiguousarray(qt_all).astype(ml_dtypes.bfloat16)
    return [
        {
            "kt": np.ascontiguousarray(kt_all[h]),
            "v": np.ascontiguousarray(v_all[h]),
            "qt": qt_all[h],
        }
        for h in range(KVH)
    ]


def _unpack_outputs(results):
    """[B,D,QCOLS] O^T + [B,128,QCOLS] lacc per core -> [B*Q, H*D]."""
    out = np.empty((B * Q, H * D), dtype=np.float32)
    for h, res in enumerate(results):
        l = res["lacc"].astype(np.float32).sum(axis=1)  # [B, QCOLS]
        o = res["out"] / l[:, None, :]  # [B, D, QCOLS]
        o = o.reshape(B, D, Q, G).transpose(0, 2, 3, 1).reshape(B * Q, G * D)
        out[:, h * G * D : (h + 1) * G * D] = o
    return out


def kernel(query, k_cache, v_cache, block_tables, seq_lens):
    query = np.asarray(query, dtype=np.float32)
    k_cache = np.asarray(k_cache, dtype=np.float32)
    v_cache = np.asarray(v_cache, dtype=np.float32)
    block_tables = np.asarray(block_tables, dtype=np.int64)
    nc, L, cb, offs = _build(np.asarray(seq_lens))
    in_maps = _pack_inputs(query, k_cache, v_cache, block_tables, L, cb, offs)
    res = run_bass_kernel_spmd(nc, in_maps, core_ids=list(range(N_CORES)))
    return _unpack_outputs(res.results)


# revision 20
# speedup vs baseline: 1.1887x; 1.0763x over previous
"""Paged GQA chunked-prefill attention for 8 Trainium2 NeuronCores.

Problem (hardcoded): B=4 seqs x Q=256 new tokens, H=32 query heads, KVH=8 kv
heads (GQA group G=4), D=128 head dim, paged KV cache of 512 blocks x 16
tokens, per-seq lengths in seq_lens (clamped to >= Q), causal masking.

Sharding: tensor-parallel over heads. Core h gets kv head h and query heads
h*4..h*4+3; block_tables/seq_lens are resolved host-side while packing the
shards; the output is all-gathered host-side over the hidden dim.

v3 design:
- All matmul operands bf16 (K, Q*SCALE, V, U); scores fp32 in PSUM.
- Softmax denominator: U accumulated per chunk into a per-seq bf16
  accumulator on VectorE; [128,1024] accumulator DMA'd out, 128-way
  partition sum + divide on the host. No reciprocal/broadcast epilogue.
- One exp ACTIVATE per chunk covering exactly the live (unmasked) columns;
  QK/PV/lacc are restricted to live columns too, so the causal-dead
  triangle costs nothing on any engine.
- Global 1-chunk software pipeline across sequences: emit score(j) then
  consume(j-1), so the PE does the next chunk's QK while ScalarE runs exp
  and sequence boundaries don't drain the pipe.
- PSUM: 2 double-buffered [128,1024] score tiles + 2 [128,1024] O
  accumulators = 8 banks exactly.
- ScalarE exp-table load and PE HAM warmup fire at t~0 on memset data;
  sequences run longest-first; O^T halves are evacuated as soon as their
  PV chain stops; lacc leaves on the gpsimd DMA queue.
"""
import math

import ml_dtypes
import numpy as np

import concourse.mybir as mybir
import concourse.tile as tile
from concourse import bacc
from concourse.bass_utils import run_bass_kernel_spmd

B, Q, H, D = 4, 256, 32, 128
KVH = 8
G = H // KVH
BLOCK = 16
NB = 128
KV = NB * BLOCK
NUM_BLOCKS = B * NB
SCALE = 1.0 / math.sqrt(D)
N_CORES = 8
CHUNK = 128
QCOLS = G * Q  # 1024 q columns per sequence per core
NHALF = 512

F32 = mybir.dt.float32
BF16 = mybir.dt.bfloat16
NEG = -1.0e9
WARMUP_MM = 6


def _plan(seq_lens):
    """Per-seq chunk counts, offsets, and boundary-chunk mask tiles."""
    L = np.maximum(np.asarray(seq_lens, dtype=np.int64), Q)
    cb = [int((int(Lb) + CHUNK - 1) // CHUNK) for Lb in L]
    offs = np.concatenate([[0], np.cumsum(cb)]).astype(int)
    masked = []  # list of (b, c, mask[128,256])
    t = np.arange(Q)
    p = np.arange(CHUNK)
    for b in range(B):
        Lb = int(L[b])
        for c in range(cb[b]):
            if c * CHUNK + CHUNK - 1 > Lb - Q:
                kvpos = c * CHUNK + p
                m = np.where(
                    kvpos[:, None] > (Lb - Q) + t[None, :], NEG, 0.0
                ).astype(np.float32)
                masked.append((b, c, m))
    return L, cb, offs, masked


def _build(seq_lens):
    L, cb, offs, masked = _plan(seq_lens)
    C = int(offs[-1])
    nmask = len(masked)
    border = sorted(range(B), key=lambda b: -cb[b])  # longest first
    order = sorted(
        range(len(masked)), key=lambda i: (border.index(masked[i][0]), masked[i][1])
    )
    masked = [masked[i] for i in order]
    mask_np = np.concatenate([m for _, _, m in masked], axis=1).astype(
        ml_dtypes.bfloat16
    )  # [128, nm*256]; 0/-1e9 are bf16-exact
    mask_idx = {(b, c): i for i, (b, c, _) in enumerate(masked)}
    identb_np = np.eye(CHUNK, dtype=ml_dtypes.bfloat16)

    nc = bacc.Bacc(
        "TRN2", target_bir_lowering=False, debug=False, num_devices=N_CORES
    )
    kt_d = nc.dram_tensor("kt", [D, C * CHUNK], BF16, kind="ExternalInput")
    v_d = nc.dram_tensor("v", [CHUNK, C * CHUNK], BF16, kind="ExternalInput")
    qt_d = nc.dram_tensor("qt", [D, B * QCOLS], BF16, kind="ExternalInput")
    out_d = nc.dram_tensor("out", [B, D, QCOLS], F32, kind="ExternalOutput")
    lacc_d = nc.dram_tensor("lacc", [B, CHUNK, QCOLS], BF16, kind="ExternalOutput")
    mask_d = nc.inline_tensor(mask_np, name="mask_const")
    identb_d = nc.inline_tensor(identb_np, name="identb_const")

    exp = mybir.ActivationFunctionType.Exp

    def states_of(b, c):
        # per half: (state, qlo_t) where qlo_t = dead leading t-cols
        out = []
        for n in range(2):
            lo = int(L[b]) - Q + n * CHUNK
            qlo = max(0, c * CHUNK - lo)
            if qlo >= CHUNK:
                out.append(("skip", CHUNK))
            elif c * CHUNK + CHUNK - 1 > lo:
                out.append(("mask", qlo))
            else:
                out.append(("clear", 0))
        return out

    with tile.TileContext(nc) as tc:
        with (
            tc.tile_pool(name="sbin", bufs=1) as sbin,
            tc.tile_pool(name="sbu", bufs=4) as sbu,
            tc.tile_pool(name="sbe", bufs=2) as sbe,
            tc.tile_pool(name="ps_s", bufs=2, space="PSUM") as ps_s,
            tc.tile_pool(name="ps_o", bufs=2, space="PSUM") as ps_o,
        ):
            # ─── t~0 warmup: exp table load + PE HAM ramp on memset data ──
            warm = sbin.tile([CHUNK, NHALF], BF16, tag="warm")
            nc.vector.memset(warm[:], 0.0)
            u_dead = sbu.tile([CHUNK, QCOLS], BF16, tag="u")
            nc.scalar.activation(u_dead[:, 0:CHUNK], warm[:, 0:CHUNK], exp)
            s_warm = ps_s.tile([CHUNK, QCOLS], F32, tag="s")
            for w in range(WARMUP_MM):
                nc.tensor.matmul(
                    s_warm[:, 0:NHALF] if w % 2 == 0 else s_warm[:, NHALF:QCOLS],
                    warm[:, 0:CHUNK],
                    warm[:],
                    start=True,
                    stop=True,
                )

            # ─── input DMAs: first-QK gates first, then streaming ─────────
            b0 = border[0]
            kt_t = [None] * B
            qt_t = [None] * B
            v_t = [None] * B
            w0 = cb[b0] * CHUNK
            o0 = offs[b0] * CHUNK
            kt_first = sbin.tile([D, w0], BF16, tag=f"kt{b0}")
            v_first = sbin.tile([CHUNK, w0], BF16, tag=f"v{b0}")
            qt_first = sbin.tile([D, QCOLS], BF16, tag=f"qt{b0}")
            # sync (HWDGE) queue: exactly what the first QK needs, smallest
            # pieces first
            # the three tensors gating the first exp go out on three parallel
            # DMA queues (sync + scalar are fast HWDGE rings; gpsimd SWDGE)
            nc.sync.dma_start(kt_first[:, 0:CHUNK], kt_d.ap()[:, o0 : o0 + CHUNK])
            nc.scalar.dma_start(
                qt_first[:, 0:NHALF],
                qt_d.ap()[:, b0 * QCOLS : b0 * QCOLS + NHALF],
            )
            nc.gpsimd.dma_start(
                qt_first[:, NHALF:QCOLS],
                qt_d.ap()[:, b0 * QCOLS + NHALF : (b0 + 1) * QCOLS],
            )
            # v chunk 0-1 (needed one pipeline stage later) on gpsimd
            head = min(2 * CHUNK, w0)
            nc.gpsimd.dma_start(v_first[:, 0:head], v_d.ap()[:, o0 : o0 + head])
            kcut, vcut = CHUNK, head
            while kcut < w0 or vcut < w0:
                khi = min(kcut + 4 * CHUNK, w0)
                if khi > kcut:
                    nc.sync.dma_start(
                        kt_first[:, kcut:khi], kt_d.ap()[:, o0 + kcut : o0 + khi]
                    )
                    kcut = khi
                vhi = min(vcut + 4 * CHUNK, w0)
                if vhi > vcut:
                    nc.sync.dma_start(
                        v_first[:, vcut:vhi], v_d.ap()[:, o0 + vcut : o0 + vhi]
                    )
                    vcut = vhi
            kt_t[b0], v_t[b0], qt_t[b0] = kt_first, v_first, qt_first

            identb = sbin.tile([CHUNK, CHUNK], BF16, tag="identb")
            nc.gpsimd.dma_start(identb[:], identb_d.ap())

            # remaining sequences stream on the gpsimd (SWDGE) queue in
            # processing order
            for b in border[1:]:
                w = cb[b] * CHUNK
                ob = offs[b] * CHUNK
                kt = sbin.tile([D, w], BF16, tag=f"kt{b}")
                vt = sbin.tile([CHUNK, w], BF16, tag=f"v{b}")
                qt = sbin.tile([D, QCOLS], BF16, tag=f"qt{b}")
                nc.gpsimd.dma_start(kt[:], kt_d.ap()[:, ob : ob + w])
                nc.gpsimd.dma_start(vt[:], v_d.ap()[:, ob : ob + w])
                nc.gpsimd.dma_start(
                    qt[:], qt_d.ap()[:, b * QCOLS : (b + 1) * QCOLS]
                )
                kt_t[b], v_t[b], qt_t[b] = kt, vt, qt

            masks = sbin.tile([CHUNK, max(nmask, 1) * Q], BF16, tag="masks")
            if nmask:
                mid = (nmask // 2) * Q
                if mid:
                    nc.sync.dma_start(masks[:, 0:mid], mask_d.ap()[:, 0:mid])
                nc.sync.dma_start(
                    masks[:, mid : nmask * Q], mask_d.ap()[:, mid : nmask * Q]
                )

            # ─── main loop: global 1-chunk software pipeline ──────────────
            jobs = [(b, c) for b in border for c in range(cb[b])]
            seq_state = {}  # b -> dict(o_ps, lacc, last_n)

            def emit_score(b, c):
                st = states_of(b, c)
                mi = mask_idx.get((b, c))
                if c == 0:
                    seq_state[b] = {
                        "o": ps_o.tile([D, QCOLS], F32, tag="o", name="o_ps"),
                        "lacc": sbin.tile(
                            [CHUNK, QCOLS], BF16, tag=f"lacc{b}", name=f"lacc{b}"
                        ),
                        "last_n": [
                            min(
                                cb[b] - 1,
                                (int(L[b]) - Q + n * CHUNK + CHUNK - 1) // CHUNK,
                            )
                            for n in range(2)
                        ],
                    }
                ss = seq_state[b]
                s = ps_s.tile([CHUNK, QCOLS], F32, tag="s")
                live_lo = QCOLS
                for n in range(2):
                    state, qlo = st[n]
                    if state == "skip":
                        continue
                    col0 = n * NHALF + qlo * G
                    live_lo = min(live_lo, col0)
                    hi = (n + 1) * NHALF
                    nc.tensor.matmul(
                        s[:, col0:hi],
                        kt_t[b][:, c * CHUNK : (c + 1) * CHUNK],
                        qt_t[b][:, col0:hi],
                        start=True,
                        stop=state == "clear",
                    )
                    if state == "mask":
                        mb = (
                            masks[
                                :,
                                mi * Q + n * CHUNK + qlo : mi * Q
                                + (n + 1) * CHUNK,
                            ]
                            .unsqueeze(2)
                            .broadcast_to([CHUNK, CHUNK - qlo, G])
                        )
                        nc.tensor.matmul(
                            s[:, col0:hi], identb[:], mb, start=False, stop=True
                        )
                u = sbu.tile([CHUNK, QCOLS], BF16, tag="u")
                live = slice(live_lo, QCOLS)
                nc.scalar.activation(u[:, live], s[:, live], exp)
                if c == 0:
                    nc.vector.tensor_copy(ss["lacc"][:, live], u[:, live])
                else:
                    nc.vector.tensor_add(
                        ss["lacc"][:, live], ss["lacc"][:, live], u[:, live]
                    )
                return st, u

            def evacuate_half(b, n, terminal=False):
                ss = seq_state[b]
                half = slice(n * NHALF, (n + 1) * NHALF)
                osb = sbe.tile([D, NHALF], F32, tag=f"osb{n}")
                if terminal:
                    # ScalarE is idle after the last exp; its copy + its own
                    # HWDGE queue keep the tail off the busy Vector/Sync FIFOs
                    nc.scalar.copy(osb[:], ss["o"][:, half])
                    nc.scalar.dma_start(out_d.ap()[b][:, half], osb[:])
                else:
                    nc.vector.tensor_copy(osb[:], ss["o"][:, half])
                    nc.sync.dma_start(out_d.ap()[b][:, half], osb[:])

            def emit_consume(b, c, st, u, terminal=False):
                ss = seq_state[b]
                for n in range(2):
                    state, qlo = st[n]
                    if state == "skip" or c > ss["last_n"][n]:
                        continue
                    col0 = n * NHALF + qlo * G
                    hi = (n + 1) * NHALF
                    nc.tensor.matmul(
                        ss["o"][:, col0:hi],
                        v_t[b][:, c * CHUNK : (c + 1) * CHUNK],
                        u[:, col0:hi],
                        start=c == 0,
                        stop=c == ss["last_n"][n],
                    )
                    if c == ss["last_n"][n]:
                        evacuate_half(b, n, terminal=terminal and n == 1)
                if c == cb[b] - 1:
                    nc.gpsimd.dma_start(lacc_d.ap()[b], ss["lacc"][:])

            pending = None
            for b, c in jobs:
                st_u = emit_score(b, c)
                if pending is not None:
                    emit_consume(*pending)
                pending = (b, c, *st_u)
            emit_consume(*pending, terminal=True)

    nc.compile()
    return nc, L, cb, offs


def _pack_inputs(query, k_cache, v_cache, block_tables, L, cb, offs):
    """Gather the paged cache and pack per-core shards in device layouts."""
    C = int(offs[-1])
    k_lin = k_cache[block_tables].reshape(B, KV, KVH, D)
    v_lin = v_cache[block_tables].reshape(B, KV, KVH, D)
    kt_all = np.zeros((KVH, D, C * CHUNK), dtype=np.float32)
    v_all = np.zeros((KVH, CHUNK, C * CHUNK), dtype=np.float32)
    for b in range(B):
        Lb, w = int(L[b]), cb[b] * CHUNK
        kk = np.zeros((w, KVH, D), dtype=np.float32)
        kk[:Lb] = k_lin[b, :Lb]
        # [w, KVH, D] -> [KVH, D, w]
        kt_all[:, :, offs[b] * CHUNK : offs[b] * CHUNK + w] = kk.transpose(
            1, 2, 0
        )
        vv = np.zeros((w, KVH, D), dtype=np.float32)
        vv[:Lb] = v_lin[b, :Lb]
        # [cb, 128, KVH, D] -> [KVH, 128, cb, D] -> [KVH, 128, w]
        v_all[:, :, offs[b] * CHUNK : offs[b] * CHUNK + w] = (
            vv.reshape(cb[b], CHUNK, KVH, D)
            .transpose(2, 1, 0, 3)
            .reshape(KVH, CHUNK, w)
        )
    # query [B,Q,H,D] -> [KVH, D, B, Q, G] (t-major, g inner); SCALE folded in
    qt_all = (
        (query * SCALE)
        .transpose(2, 3, 0, 1)
        .reshape(KVH, G, D, B, Q)
        .transpose(0, 2, 3, 4, 1)
        .reshape(KVH, D, B * QCOLS)
    )
    kt_all = kt_all.astype(ml_dtypes.bfloat16)
    v_all = v_all.astype(ml_dtypes.bfloat16)
    qt_all = np.ascontiguousarray(qt_all).astype(ml_dtypes.bfloat16)
    return [
        {
            "kt": np.ascontiguousarray(kt_all[h]),
            "v": np.ascontiguousarray(v_all[h]),
            "qt": qt_all[h],
        }
        for h in range(KVH)
    ]


def _unpack_outputs(results):
    """[B,D,QCOLS] O^T + [B,128,QCOLS] lacc per core -> [B*Q, H*D]."""
    out = np.empty((B * Q, H * D), dtype=np.float32)
    for h, res in enumerate(results):
        l = res["lacc"].astype(np.float32).sum(axis=1)  # [B, QCOLS]
        o = res["out"] / l[:, None, :]  # [B, D, QCOLS]
        o = o.reshape(B, D, Q, G).transpose(0, 2, 3, 1).reshape(B * Q, G * D)
        out[:, h * G * D : (h + 1) * G * D] = o
    return out


def kernel(query, k_cache, v_cache, block_tables, seq_lens):
    query = np.asarray(query, dtype=np.float32)
    k_cache = np.asarray(k_cache, dtype=np.float32)
    v_cache = np.asarray(v_cache, dtype=np.float32)
    block_tables = np.asarray(block_tables, dtype=np.int64)
    nc, L, cb, offs = _build(np.asarray(seq_lens))
    in_maps = _pack_inputs(query, k_cache, v_cache, block_tables, L, cb, offs)
    res = run_bass_kernel_spmd(nc, in_maps, core_ids=list(range(N_CORES)))
    return _unpack_outputs(res.results)


# revision 22
# speedup vs baseline: 1.2431x; 1.0457x over previous
"""Paged GQA chunked-prefill attention for 8 Trainium2 NeuronCores.

Problem (hardcoded): B=4 seqs x Q=256 new tokens, H=32 query heads, KVH=8 kv
heads (GQA group G=4), D=128 head dim, paged KV cache of 512 blocks x 16
tokens, per-seq lengths in seq_lens (clamped to >= Q), causal masking.

Sharding: tensor-parallel over heads. Core h gets kv head h and query heads
h*4..h*4+3; block_tables/seq_lens are resolved host-side while packing the
shards; the output is all-gathered host-side over the hidden dim.

v3 design:
- All matmul operands bf16 (K, Q*SCALE, V, U); scores fp32 in PSUM.
- Softmax denominator: U accumulated per chunk into a per-seq bf16
  accumulator on VectorE; [128,1024] accumulator DMA'd out, 128-way
  partition sum + divide on the host. No reciprocal/broadcast epilogue.
- One exp ACTIVATE per chunk covering exactly the live (unmasked) columns;
  QK/PV/lacc are restricted to live columns too, so the causal-dead
  triangle costs nothing on any engine.
- Global 1-chunk software pipeline across sequences: emit score(j) then
  consume(j-1), so the PE does the next chunk's QK while ScalarE runs exp
  and sequence boundaries don't drain the pipe.
- PSUM: 2 double-buffered [128,1024] score tiles + 2 [128,1024] O
  accumulators = 8 banks exactly.
- ScalarE exp-table load and PE HAM warmup fire at t~0 on memset data;
  sequences run longest-first; O^T halves are evacuated as soon as their
  PV chain stops; lacc leaves on the gpsimd DMA queue.
"""
import math

import ml_dtypes
import numpy as np

import concourse.mybir as mybir
import concourse.tile as tile
from concourse import bacc
from concourse.bass_utils import run_bass_kernel_spmd

B, Q, H, D = 4, 256, 32, 128
KVH = 8
G = H // KVH
BLOCK = 16
NB = 128
KV = NB * BLOCK
NUM_BLOCKS = B * NB
SCALE = 1.0 / math.sqrt(D)
N_CORES = 8
CHUNK = 128
QCOLS = G * Q  # 1024 q columns per sequence per core
NHALF = 512

F32 = mybir.dt.float32
BF16 = mybir.dt.bfloat16
NEG = -1.0e9
WARMUP_MM = 6


def _plan(seq_lens):
    """Per-seq chunk counts, offsets, and boundary-chunk mask tiles."""
    L = np.maximum(np.asarray(seq_lens, dtype=np.int64), Q)
    cb = [int((int(Lb) + CHUNK - 1) // CHUNK) for Lb in L]
    offs = np.concatenate([[0], np.cumsum(cb)]).astype(int)
    masked = []  # list of (b, c, mask[128,256])
    t = np.arange(Q)
    p = np.arange(CHUNK)
    for b in range(B):
        Lb = int(L[b])
        for c in range(cb[b]):
            if c * CHUNK + CHUNK - 1 > Lb - Q:
                kvpos = c * CHUNK + p
                m = np.where(
                    kvpos[:, None] > (Lb - Q) + t[None, :], NEG, 0.0
                ).astype(np.float32)
                masked.append((b, c, m))
    return L, cb, offs, masked


def _build(seq_lens):
    L, cb, offs, masked = _plan(seq_lens)
    C = int(offs[-1])
    nmask = len(masked)
    border = sorted(range(B), key=lambda b: -cb[b])  # longest first
    order = sorted(
        range(len(masked)), key=lambda i: (border.index(masked[i][0]), masked[i][1])
    )
    masked = [masked[i] for i in order]
    mask_np = np.concatenate([m for _, _, m in masked], axis=1).astype(
        ml_dtypes.bfloat16
    )  # [128, nm*256]; 0/-1e9 are bf16-exact
    mask_idx = {(b, c): i for i, (b, c, _) in enumerate(masked)}
    identb_np = np.eye(CHUNK, dtype=ml_dtypes.bfloat16)

    nc = bacc.Bacc(
        "TRN2", target_bir_lowering=False, debug=False, num_devices=N_CORES
    )
    kt_d = nc.dram_tensor("kt", [D, C * CHUNK], BF16, kind="ExternalInput")
    v_d = nc.dram_tensor("v", [CHUNK, C * CHUNK], BF16, kind="ExternalInput")
    qt_d = nc.dram_tensor("qt", [D, B * QCOLS], BF16, kind="ExternalInput")
    out_d = nc.dram_tensor("out", [B, D, QCOLS], F32, kind="ExternalOutput")
    lacc_d = nc.dram_tensor("lacc", [B, CHUNK, QCOLS], BF16, kind="ExternalOutput")
    mask_d = nc.inline_tensor(mask_np, name="mask_const")
    identb_d = nc.inline_tensor(identb_np, name="identb_const")

    exp = mybir.ActivationFunctionType.Exp

    def states_of(b, c):
        # per half: (state, qlo_t) where qlo_t = dead leading t-cols
        out = []
        for n in range(2):
            lo = int(L[b]) - Q + n * CHUNK
            qlo = max(0, c * CHUNK - lo)
            if qlo >= CHUNK:
                out.append(("skip", CHUNK))
            elif c * CHUNK + CHUNK - 1 > lo:
                out.append(("mask", qlo))
            else:
                out.append(("clear", 0))
        return out

    with tile.TileContext(nc) as tc:
        with (
            tc.tile_pool(name="sbin", bufs=1) as sbin,
            tc.tile_pool(name="sbu", bufs=4) as sbu,
            tc.tile_pool(name="sbe", bufs=2) as sbe,
            tc.tile_pool(name="ps_s", bufs=3, space="PSUM") as ps_s,
            tc.tile_pool(name="ps_o", bufs=1, space="PSUM") as ps_o,
        ):
            # ─── t~0 warmup: exp table load + PE HAM ramp on memset data ──
            warm = sbin.tile([CHUNK, NHALF], BF16, tag="warm")
            nc.vector.memset(warm[:], 0.0)
            u_dead = sbu.tile([CHUNK, QCOLS], BF16, tag="u")
            nc.scalar.activation(u_dead[:, 0:CHUNK], warm[:, 0:CHUNK], exp)
            s_warm = ps_s.tile([CHUNK, QCOLS], F32, tag="s")
            for w in range(WARMUP_MM):
                nc.tensor.matmul(
                    s_warm[:, 0:NHALF] if w % 2 == 0 else s_warm[:, NHALF:QCOLS],
                    warm[:, 0:CHUNK],
                    warm[:],
                    start=True,
                    stop=True,
                )

            # ─── input DMAs: first-QK gates first, then streaming ─────────
            b0 = border[0]
            kt_t = [None] * B
            qt_t = [None] * B
            v_t = [None] * B
            w0 = cb[b0] * CHUNK
            o0 = offs[b0] * CHUNK
            kt_first = sbin.tile([D, w0], BF16, tag=f"kt{b0}")
            v_first = sbin.tile([CHUNK, w0], BF16, tag=f"v{b0}")
            qt_first = sbin.tile([D, QCOLS], BF16, tag=f"qt{b0}")
            # sync (HWDGE) queue: exactly what the first QK needs, smallest
            # pieces first
            # the three tensors gating the first exp go out on three parallel
            # DMA queues (sync + scalar are fast HWDGE rings; gpsimd SWDGE)
            nc.sync.dma_start(kt_first[:, 0:CHUNK], kt_d.ap()[:, o0 : o0 + CHUNK])
            nc.scalar.dma_start(
                qt_first[:, 0:NHALF],
                qt_d.ap()[:, b0 * QCOLS : b0 * QCOLS + NHALF],
            )
            nc.gpsimd.dma_start(
                qt_first[:, NHALF:QCOLS],
                qt_d.ap()[:, b0 * QCOLS + NHALF : (b0 + 1) * QCOLS],
            )
            # v chunk 0-1 (needed one pipeline stage later) on gpsimd
            head = min(2 * CHUNK, w0)
            nc.gpsimd.dma_start(v_first[:, 0:head], v_d.ap()[:, o0 : o0 + head])
            kcut, vcut = CHUNK, head
            while kcut < w0 or vcut < w0:
                khi = min(kcut + 4 * CHUNK, w0)
                if khi > kcut:
                    nc.sync.dma_start(
                        kt_first[:, kcut:khi], kt_d.ap()[:, o0 + kcut : o0 + khi]
                    )
                    kcut = khi
                vhi = min(vcut + 4 * CHUNK, w0)
                if vhi > vcut:
                    nc.sync.dma_start(
                        v_first[:, vcut:vhi], v_d.ap()[:, o0 + vcut : o0 + vhi]
                    )
                    vcut = vhi
            kt_t[b0], v_t[b0], qt_t[b0] = kt_first, v_first, qt_first

            identb = sbin.tile([CHUNK, CHUNK], BF16, tag="identb")
            nc.gpsimd.dma_start(identb[:], identb_d.ap())

            # remaining sequences stream on the gpsimd (SWDGE) queue in
            # processing order
            for b in border[1:]:
                w = cb[b] * CHUNK
                ob = offs[b] * CHUNK
                kt = sbin.tile([D, w], BF16, tag=f"kt{b}")
                vt = sbin.tile([CHUNK, w], BF16, tag=f"v{b}")
                qt = sbin.tile([D, QCOLS], BF16, tag=f"qt{b}")
                nc.gpsimd.dma_start(kt[:], kt_d.ap()[:, ob : ob + w])
                nc.gpsimd.dma_start(vt[:], v_d.ap()[:, ob : ob + w])
                nc.gpsimd.dma_start(
                    qt[:], qt_d.ap()[:, b * QCOLS : (b + 1) * QCOLS]
                )
                kt_t[b], v_t[b], qt_t[b] = kt, vt, qt

            masks = sbin.tile([CHUNK, max(nmask, 1) * Q], BF16, tag="masks")
            if nmask:
                mid = (nmask // 2) * Q
                if mid:
                    nc.sync.dma_start(masks[:, 0:mid], mask_d.ap()[:, 0:mid])
                nc.sync.dma_start(
                    masks[:, mid : nmask * Q], mask_d.ap()[:, mid : nmask * Q]
                )

            # ─── main loop: global 1-chunk software pipeline ──────────────
            jobs = [(b, c) for b in border for c in range(cb[b])]
            seq_state = {}  # b -> dict(o_ps, lacc, last_n)

            def emit_score(b, c):
                st = states_of(b, c)
                mi = mask_idx.get((b, c))
                if c == 0:
                    seq_state[b] = {
                        "o": ps_o.tile([D, QCOLS], F32, tag="o", name="o_ps"),
                        "lacc": sbin.tile(
                            [CHUNK, QCOLS], BF16, tag=f"lacc{b}", name=f"lacc{b}"
                        ),
                        "last_n": [
                            min(
                                cb[b] - 1,
                                (int(L[b]) - Q + n * CHUNK + CHUNK - 1) // CHUNK,
                            )
                            for n in range(2)
                        ],
                    }
                ss = seq_state[b]
                s = ps_s.tile([CHUNK, QCOLS], F32, tag="s")
                live_lo = QCOLS
                for n in range(2):
                    state, qlo = st[n]
                    if state == "skip":
                        continue
                    col0 = n * NHALF + qlo * G
                    live_lo = min(live_lo, col0)
                    hi = (n + 1) * NHALF
                    nc.tensor.matmul(
                        s[:, col0:hi],
                        kt_t[b][:, c * CHUNK : (c + 1) * CHUNK],
                        qt_t[b][:, col0:hi],
                        start=True,
                        stop=state == "clear",
                    )
                    if state == "mask":
                        # the mask is nonzero only for t < t_hi (the causal
                        # boundary is <=128 wide) — except in the last chunk,
                        # where zero-padded kv rows mask every live t
                        lo = int(L[b]) - Q + n * CHUNK
                        if c == cb[b] - 1 and int(L[b]) % CHUNK:
                            thi = CHUNK
                        else:
                            thi = min(CHUNK, c * CHUNK + CHUNK - lo)
                        mb = (
                            masks[
                                :,
                                mi * Q + n * CHUNK + qlo : mi * Q
                                + n * CHUNK
                                + thi,
                            ]
                            .unsqueeze(2)
                            .broadcast_to([CHUNK, thi - qlo, G])
                        )
                        nc.tensor.matmul(
                            s[:, col0 : n * NHALF + thi * G],
                            identb[:],
                            mb,
                            start=False,
                            stop=True,
                        )
                u = sbu.tile([CHUNK, QCOLS], BF16, tag="u")
                live = slice(live_lo, QCOLS)
                nc.scalar.activation(u[:, live], s[:, live], exp)
                if c == 0:
                    nc.vector.tensor_copy(ss["lacc"][:, live], u[:, live])
                else:
                    nc.vector.tensor_add(
                        ss["lacc"][:, live], ss["lacc"][:, live], u[:, live]
                    )
                return st, u

            def evacuate_half(b, n, terminal=False):
                ss = seq_state[b]
                half = slice(n * NHALF, (n + 1) * NHALF)
                osb = sbe.tile([D, NHALF], F32, tag=f"osb{n}")
                if terminal:
                    # ScalarE is idle after the last exp; its copy + its own
                    # HWDGE queue keep the tail off the busy Vector/Sync FIFOs
                    nc.scalar.copy(osb[:], ss["o"][:, half])
                    nc.scalar.dma_start(out_d.ap()[b][:, half], osb[:])
                else:
                    nc.vector.tensor_copy(osb[:], ss["o"][:, half])
                    nc.sync.dma_start(out_d.ap()[b][:, half], osb[:])

            def emit_consume(b, c, st, u, terminal=False):
                ss = seq_state[b]
                for n in range(2):
                    state, qlo = st[n]
                    if state == "skip" or c > ss["last_n"][n]:
                        continue
                    col0 = n * NHALF + qlo * G
                    hi = (n + 1) * NHALF
                    nc.tensor.matmul(
                        ss["o"][:, col0:hi],
                        v_t[b][:, c * CHUNK : (c + 1) * CHUNK],
                        u[:, col0:hi],
                        start=c == 0,
                        stop=c == ss["last_n"][n],
                    )
                    if c == ss["last_n"][n]:
                        evacuate_half(b, n, terminal=terminal and n == 1)
                if c == cb[b] - 1:
                    nc.gpsimd.dma_start(lacc_d.ap()[b], ss["lacc"][:])

            pending = None
            for b, c in jobs:
                st_u = emit_score(b, c)
                if pending is not None:
                    emit_consume(*pending)
                pending = (b, c, *st_u)
            emit_consume(*pending, terminal=True)

    nc.compile()
    return nc, L, cb, offs


def _pack_inputs(query, k_cache, v_cache, block_tables, L, cb, offs):
    """Gather the paged cache and pack per-core shards in device layouts."""
    C = int(offs[-1])
    k_lin = k_cache[block_tables].reshape(B, KV, KVH, D)
    v_lin = v_cache[block_tables].reshape(B, KV, KVH, D)
    kt_all = np.zeros((KVH, D, C * CHUNK), dtype=np.float32)
    v_all = np.zeros((KVH, CHUNK, C * CHUNK), dtype=np.float32)
    for b in range(B):
        Lb, w = int(L[b]), cb[b] * CHUNK
        kk = np.zeros((w, KVH, D), dtype=np.float32)
        kk[:Lb] = k_lin[b, :Lb]
        # [w, KVH, D] -> [KVH, D, w]
        kt_all[:, :, offs[b] * CHUNK : offs[b] * CHUNK + w] = kk.transpose(
            1, 2, 0
        )
        vv = np.zeros((w, KVH, D), dtype=np.float32)
        vv[:Lb] = v_lin[b, :Lb]
        # [cb, 128, KVH, D] -> [KVH, 128, cb, D] -> [KVH, 128, w]
        v_all[:, :, offs[b] * CHUNK : offs[b] * CHUNK + w] = (
            vv.reshape(cb[b], CHUNK, KVH, D)
            .transpose(2, 1, 0, 3)
            .reshape(KVH, CHUNK, w)
        )
    # query [B,Q,H,D] -> [KVH, D, B, Q, G] (t-major, g inner); SCALE folded in
    qt_all = (
        (query * SCALE)
        .transpose(2, 3, 0, 1)
        .reshape(KVH, G, D, B, Q)
        .transpose(0, 2, 3, 4, 1)
        .reshape(KVH, D, B * QCOLS)
    )
    kt_all = kt_all.astype(ml_dtypes.bfloat16)
    v_all = v_all.astype(ml_dtypes.bfloat16)
    qt_all = np.ascontiguousarray(qt_all).astype(ml_dtypes.bfloat16)
    return [
        {
            "kt": np.ascontiguousarray(kt_all[h]),
            "v": np.ascontiguousarray(v_all[h]),
            "qt": qt_all[h],
        }
        for h in range(KVH)
    ]


def _unpack_outputs(results):
    """[B,D,QCOLS] O^T + [B,128,QCOLS] lacc per core -> [B*Q, H*D]."""
    out = np.empty((B * Q, H * D), dtype=np.float32)
    for h, res in enumerate(results):
        l = res["lacc"].astype(np.float32).sum(axis=1)  # [B, QCOLS]
        o = res["out"] / l[:, None, :]  # [B, D, QCOLS]
        o = o.reshape(B, D, Q, G).transpose(0, 2, 3, 1).reshape(B * Q, G * D)
        out[:, h * G * D : (h + 1) * G * D] = o
    return out


def kernel(query, k_cache, v_cache, block_tables, seq_lens):
    query = np.asarray(query, dtype=np.float32)
    k_cache = np.asarray(k_cache, dtype=np.float32)
    v_cache = np.asarray(v_cache, dtype=np.float32)
    block_tables = np.asarray(block_tables, dtype=np.int64)
    nc, L, cb, offs = _build(np.asarray(seq_lens))
    in_maps = _pack_inputs(query, k_cache, v_cache, block_tables, L, cb, offs)
    res = run_bass_kernel_spmd(nc, in_maps, core_ids=list(range(N_CORES)))
    return _unpack_outputs(res.results)
